# revision 7
# baseline (speedup 1.0000x reference)
"""TT-embedding lookup kernel for 8 trn2 NeuronCores.

Strategy: the three TT cores are tiny (~2 MB) and the full expanded embedding
table is exactly the size of the output (1e6 x 128 f32 = 512 MB, N = 1<<20
uniform random indices -> ~63% of table rows are actually needed, every row
w.h.p. per 8-core slice). Each core materializes the table slice for 1/8 of
the i0 range (13 of 100 i0 values) with dense, PE-friendly matmuls:

  stage 1: AB[(i0,q0), (i1,q1,r2)] = A^T.T @ B^T          (K=r1=32)
  stage 2: ABt = transpose(AB) -> [(q1,r2), (i1,i0,q0)]   (PE transpose)
  stage 3: OUT[(pair,q0), (i2,q2)] = ABt.T @ C2           (K=r2=32, per q1)

Stage-3 tiles [128, 800] are dumped sequentially to HBM (pure line-rate DMA,
no strided writes). The host reassembles the per-core value tables and
performs the final index->row gather (the unshard step).

Shapes hardcoded from the problem spec:
  P=(100,100,100), Q=(4,4,8), R=(1,32,32,1), N=1<<20.
"""

import numpy as np

P0, P1, P2 = 100, 100, 100
Q0, Q1, Q2 = 4, 4, 8
R1, R2 = 32, 32
NCORES = 8

# i0 ranges per core: first 4 cores take 13 i0 values, last 4 take 12 (=100).
NI0 = [13, 13, 13, 13, 12, 12, 12, 12]
I0_BASE = np.cumsum([0] + NI0)[:-1]  # [0,13,26,39,52,64,76,88]
NI0_PAD = 13                      # padded i0 count per core (uniform program)
NPAIR = P1 * NI0_PAD              # 1300
NPAIR_PAD = 1312                  # 41 blocks of 32 pairs
NPB = NPAIR_PAD // 32             # 41 pair-blocks
WCOLS = P2 * Q2                   # 800 output cols (i2, q2)

_cache = {}


def _build_program(step3_dtype_name="float32r"):
    from concourse import bacc
    import concourse.mybir as mybir
    from concourse.tile import TileContext

    f32 = mybir.dt.float32
    f32r = getattr(mybir.dt, step3_dtype_name)

    nc = bacc.Bacc("TRN2", target_bir_lowering=False, debug=False,
                   num_devices=NCORES)

    at = nc.dram_tensor("at", [R1, NI0_PAD * Q0], f32, kind="ExternalInput")
    bt = nc.dram_tensor("bt", [R1, P1 * Q1 * R2], f32, kind="ExternalInput")
    c2r = nc.dram_tensor("c2r", [128, WCOLS], f32, kind="ExternalInput")
    ident = nc.dram_tensor("ident", [128, 128], f32, kind="ExternalInput")
    out = nc.dram_tensor("out", [Q1, NPB, 128, WCOLS], f32, kind="ExternalOutput")

    ACOLS = NI0_PAD * Q0          # 52
    BCOLS = P1 * Q1 * R2          # 12800

    with TileContext(nc) as tc:
        with tc.tile_pool(name="const", bufs=1) as cpool, \
             tc.tile_pool(name="abp", bufs=1) as abpool, \
             tc.tile_pool(name="abtp", bufs=1) as abtpool, \
             tc.tile_pool(name="ps1", bufs=2, space="PSUM") as ps1pool, \
             tc.tile_pool(name="ps2", bufs=2, space="PSUM") as ps2pool, \
             tc.tile_pool(name="ps3", bufs=4, space="PSUM") as ps3pool, \
             tc.tile_pool(name="osb", bufs=3) as opool:

            at_sb = cpool.tile([R1, ACOLS], f32, tag="at")
            bt_sb = cpool.tile([R1, BCOLS], f32, tag="bt")
            c2_sb = cpool.tile([128, WCOLS], f32r, tag="c2")
            id_sb = cpool.tile([128, 128], f32, tag="id")
            nc.sync.dma_start(out=at_sb[:], in_=at[:])
            nc.sync.dma_start(out=bt_sb[:], in_=bt[:])
            nc.gpsimd.dma_start(out=c2_sb[:], in_=c2r[:])
            nc.sync.dma_start(out=id_sb[:], in_=ident[:])

            # stage 1: AB [52, 12800] = at.T @ bt, fp32 exact
            ab_sb = abpool.tile([ACOLS, BCOLS], f32, tag="ab")
            for j in range(BCOLS // 512):
                ps = ps1pool.tile([ACOLS, 512], f32, tag="ps1")
                nc.tensor.matmul(ps[:], at_sb[:], bt_sb[:, j * 512:(j + 1) * 512],
                                 start=True, stop=True)
                nc.any.tensor_copy(ab_sb[:, j * 512:(j + 1) * 512], ps[:])

            # stage 2: ABt [(q1,r2)=128, (i1, i0, q0)=5248] via PE transposes
            abt_sb = abtpool.tile([128, NPAIR_PAD * Q0], f32r, tag="abt")
            for i1 in range(P1):
                pst = ps2pool.tile([128, ACOLS], f32, tag="ps2")
                nc.tensor.transpose(pst[:], ab_sb[:, i1 * 128:(i1 + 1) * 128],
                                    id_sb[:ACOLS, :ACOLS])
                nc.any.tensor_copy(
                    abt_sb[:, i1 * ACOLS:(i1 + 1) * ACOLS], pst[:])

            # stage 3: per (q1, pair-block): [128,800] = ABt_blk.T @ C2_blk
            for q1 in range(Q1):
                for pb in range(NPB):
                    osb = opool.tile([128, WCOLS], f32, tag="osb")
                    for h in range(2):
                        ps3 = ps3pool.tile([128, WCOLS // 2], f32, tag="ps3")
                        nc.tensor.matmul(
                            ps3[:],
                            abt_sb[32 * q1:32 * (q1 + 1),
                                   pb * 128:(pb + 1) * 128],
                            c2_sb[32 * q1:32 * (q1 + 1),
                                  h * 400:(h + 1) * 400],
                            start=True, stop=True,
                            tile_position=(32 * q1, 0),
                        )
                        nc.any.tensor_copy(osb[:, h * 400:(h + 1) * 400], ps3[:])
                    nc.sync.dma_start(out=out[q1, pb], in_=osb[:])
    nc.finalize()
    return nc


def _host_inputs(core0, core1, core2, k):
    base, ni0 = I0_BASE[k], NI0[k]
    i0s = np.arange(base, base + ni0)
    i0s = np.concatenate([i0s, np.repeat(i0s[-1:], NI0_PAD - ni0)])
    at = core0[i0s].reshape(NI0_PAD, Q0, R1).transpose(2, 0, 1).reshape(
        R1, NI0_PAD * Q0)
    bt = core1.reshape(P1, R1, Q1 * R2).transpose(1, 0, 2).reshape(
        R1, P1 * Q1 * R2)
    c2 = core2.reshape(P2, R2, Q2).transpose(1, 0, 2).reshape(R2, P2 * Q2)
    c2r = np.tile(c2, (4, 1))
    ident = np.eye(128, dtype=np.float32)
    return {
        "at": np.ascontiguousarray(at, np.float32),
        "bt": np.ascontiguousarray(bt, np.float32),
        "c2r": np.ascontiguousarray(c2r, np.float32),
        "ident": ident,
    }


def run_device(core0, core1, core2, trace=False, step3_dtype="float32r"):
    from concourse.bass_utils import run_bass_kernel_spmd
    key = step3_dtype
    if key not in _cache:
        _cache[key] = _build_program(step3_dtype)
    nc = _cache[key]
    in_maps = [_host_inputs(core0, core1, core2, k) for k in range(NCORES)]
    res = run_bass_kernel_spmd(nc, in_maps, core_ids=list(range(NCORES)),
                               trace=trace)
    return res


def _reassemble(raw):
    # raw: [q1, pb, (ps,q0), (w,q2)] -> value table [pair*100 + i2, 128]
    vt = raw.reshape(Q1, NPB, 32, Q0, P2, Q2)
    vt = vt.transpose(1, 2, 4, 3, 0, 5)          # [pb, ps, w, q0, q1, q2]
    return np.ascontiguousarray(vt).reshape(NPAIR_PAD * P2, Q0 * Q1 * Q2)


def kernel(core0, core1, core2, indices):
    core0 = np.asarray(core0, np.float32)
    core1 = np.asarray(core1, np.float32)
    core2 = np.asarray(core2, np.float32)
    idx = np.asarray(indices)

    res = run_device(core0, core1, core2)
    vts = [_reassemble(r["out"]) for r in res.results]

    i0 = (idx // (P1 * P2)).astype(np.int64)
    i1 = ((idx // P2) % P1).astype(np.int64)
    i2 = (idx % P2).astype(np.int64)
    core_of = np.zeros(P0, np.int64)
    for k in range(NCORES):
        core_of[I0_BASE[k]:I0_BASE[k] + NI0[k]] = k
    ck = core_of[i0]

    out = np.empty((idx.shape[0], Q0 * Q1 * Q2), np.float32)
    for k in range(NCORES):
        m = ck == k
        rows = (i1[m] * NI0_PAD + (i0[m] - I0_BASE[k])) * P2 + i2[m]
        out[m] = vts[k][rows]
    return out
